# revision 16
# baseline (speedup 1.0000x reference)
"""Embedding gather (DirectCXLEmbedding) on 8 TRN2 NeuronCores.

Design (vocab-sharded + dedup + int16 SWDGE gather):

1. Vocab (table) sharding: core i owns table rows [i*125000, (i+1)*125000)
   and handles the indices landing in its shard (~102,400 of the global
   819,200 for uniform inputs).  The host routes indices to owner cores by
   sorting them once; the "all-to-all" of classic vocab-sharded embeddings
   is free because kernel() owns full inputs and outputs anyway.  Each core
   only receives its 32 MB table slice.

2. Dedup: at 0.82 draws/row, ~32% of a core's sorted indices are duplicates.
   The device gathers each unique row once (~69,900 rows/core); the host
   expands duplicates during the same fancy-index that inverts the sort.
   Device traffic drops ~32% in both directions (~18 MB read + ~18 MB
   written per core).

3. Gather engine: GPSIMD `dma_gather` (InstDMAGatherAnt, SWDGE) gathers up
   to 1024 rows per instruction (HW limit found empirically; >1024 crashes
   the device) by int16 index.  The host cuts each core's sorted unique
   local indices into 70 chunks of <=1024; chunk c reads from a STATIC
   32,768-row window based at the expected rank-quantile minus margin, so
   chunk-local indices fit int16 with large slack.  Out-of-window indices
   (non-uniform inputs) spill to a host-side numpy gather — zero spills for
   the target workload.  Sorted unique indices also make the "random"
   gather nearly sequential in HBM (mean gap 1.8 rows).

4. Device pipeline: per chunk, one dma_gather (valid count via a runtime
   register loaded from an input tensor) into an SBUF staging slot, then a
   contiguous HWDGE store from SP.  Gathers (GPSIMD/SWDGE) and stores
   (SP/HWDGE) overlap; staging slots rotate over NBUF per-slot semaphore
   pairs (a DMA's "+16" is 16 independent +1s from the SDMA engines, so a
   semaphore is only safely waitable with a single DMA in flight on it).
"""

import numpy as np

# Problem constants (hardcoded per harness contract).
B, L = 16384, 50
V, D = 1_000_000, 64
N_CORES = 8
P = 128
N_FLAT = B * L                            # 819,200 total gathers

SHARD = V // N_CORES                      # 125,000 table rows per core
CHUNK = 1024                              # max num_idxs per dma_gather
NCH = 70                                  # chunks per core (71,680 capacity)
CAPACITY = NCH * CHUNK
S = CHUNK // 16                           # int16 idx columns per chunk (64)
J = CHUNK // 128                          # gathered columns per chunk (8)
WIN = 1 << 15                             # int16 window (32768 rows)
MARGIN = 6_000
NBUF = 16                                 # staging slots

# Static per-chunk window bases within a shard: chunk c holds sorted unique
# ranks [c*1024, (c+1)*1024); with ~69,900 unique draws over 125,000 rows the
# value at unique rank r concentrates near r*1.788.
_EXP_UNIQUE = 69_900
BASES = np.clip(
    (np.arange(NCH) * CHUNK * SHARD) // _EXP_UNIQUE - MARGIN,
    0,
    SHARD - WIN,
).astype(np.int64)


def _build_module():
    from contextlib import ExitStack

    import concourse.bacc as bacc
    import concourse.mybir as mybir

    nc = bacc.Bacc()

    idxs = nc.dram_tensor("idxs", [P, NCH * S], mybir.dt.int16, kind="ExternalInput")
    counts = nc.dram_tensor("counts", [1, NCH], mybir.dt.int32, kind="ExternalInput")
    weight = nc.dram_tensor("weight", [SHARD, D], mybir.dt.float32, kind="ExternalInput")
    out = nc.dram_tensor("out", [NCH, P, J * D], mybir.dt.float32, kind="ExternalOutput")

    with ExitStack() as ctx:
        idx_sb = ctx.enter_context(nc.sbuf_tensor([P, NCH * S], mybir.dt.int16))
        cnt_sb = ctx.enter_context(nc.sbuf_tensor([1, NCH], mybir.dt.int32))
        stage = ctx.enter_context(
            nc.sbuf_tensor([P, NBUF * J * D], mybir.dt.float32)
        )
        ld_sem = ctx.enter_context(nc.semaphore("ld_sem"))
        ig_sems = [
            ctx.enter_context(nc.semaphore(f"ig{t}")) for t in range(NBUF)
        ]
        st_sems = [
            ctx.enter_context(nc.semaphore(f"st{t}")) for t in range(NBUF)
        ]
        vz_sem = ctx.enter_context(nc.semaphore("vz_sem"))
        cnt_reg = ctx.enter_context(nc.gpsimd.register("cnt_reg"))
        block = ctx.enter_context(nc.Block())

        @block.vector
        def _(v):
            # init staging once: -1-padded gather lanes are skipped by the HW,
            # so stores would otherwise move uninitialized SBUF
            v.memset(stage[:], 0.0).then_inc(vz_sem, 1)

        @block.gpsimd
        def _(g):
            g.wait_ge(vz_sem, 1)
            g.dma_start(out=idx_sb[:], in_=idxs[:]).then_inc(ld_sem, 16)
            g.wait_ge(ld_sem, 16)
            g.dma_start(out=cnt_sb[:], in_=counts[:]).then_inc(ld_sem, 16)
            g.wait_ge(ld_sem, 32)
            for c in range(NCH):
                slot = c % NBUF
                if c >= NBUF:
                    # staging slot must have been stored out (same-lane store)
                    g.wait_ge(st_sems[slot], 16 * (c // NBUF))
                g.reg_load(cnt_reg, cnt_sb[0:1, c:c + 1])
                base = int(BASES[c])
                g.dma_gather(
                    out_ap=stage[:, slot * J * D:(slot + 1) * J * D].rearrange(
                        "p (j d) -> p j d", d=D
                    ),
                    in_ap=weight[base:base + WIN, :],
                    idxs_ap=idx_sb[:, c * S:(c + 1) * S],
                    num_idxs=CHUNK,
                    num_idxs_reg=cnt_reg,
                    elem_size=D,
                ).then_inc(ig_sems[slot], 16)

        @block.sync
        def _(s):
            for c in range(NCH):
                slot = c % NBUF
                s.wait_ge(ig_sems[slot], 16 * (c // NBUF + 1))
                s.dma_start(
                    out=out[c, :, :],
                    in_=stage[:, slot * J * D:(slot + 1) * J * D],
                ).then_inc(st_sems[slot], 16)
            for c in range(NCH - NBUF, NCH):
                slot = c % NBUF
                s.wait_ge(st_sems[slot], 16 * (c // NBUF + 1))

    nc.compile()
    return nc


_NC_CACHE = None


def _prep_core(local_sorted: np.ndarray):
    """Prep one core: local_sorted is this core's sorted shard-local indices
    (with duplicates).  Returns (idx16, counts, u_rank, uvals, valid_u)."""
    n = len(local_sorted)
    if n == 0:
        idx16 = np.full((P, NCH * S), -1, dtype=np.int16)
        idx16[:16, 0] = 0
        counts = np.ones((1, NCH), dtype=np.int32)
        return idx16, counts, np.empty(0, np.int64), np.empty(0, np.int64), \
            np.empty(0, bool)

    newv = np.empty(n, dtype=bool)
    newv[0] = True
    np.not_equal(local_sorted[1:], local_sorted[:-1], out=newv[1:])
    u_rank = np.cumsum(newv) - 1                         # sorted rank -> unique rank
    uvals = local_sorted[newv]                           # sorted unique values
    n_u = len(uvals)

    pad = np.full(CAPACITY, -1, dtype=np.int64)
    take = min(n_u, CAPACITY)
    pad[:take] = uvals[:take]
    chunks = pad.reshape(NCH, CHUNK)
    rel = chunks - BASES[:, None]
    in_win = (rel >= 0) & (rel < WIN) & (chunks >= 0)

    buf = np.full((NCH, CHUNK), -1, dtype=np.int16)
    counts = np.empty(NCH, dtype=np.int32)
    nval = in_win.sum(axis=1)
    for c in range(NCH):
        buf[c, :nval[c]] = rel[c][in_win[c]].astype(np.int16)
        if nval[c] == 0:
            buf[c, 0] = 0                                # dummy; discarded
        counts[c] = max(int(nval[c]), 1)

    # valid_u[u] = True iff unique rank u was gathered on device
    valid_u = np.zeros(n_u, dtype=bool)
    valid_u[:take] = in_win.reshape(-1)[:take]

    # wrap: slot s -> [s % 16, s // 16], then replicate to 128 partitions
    idx16 = buf.reshape(NCH, S, 16).transpose(0, 2, 1)   # [NCH, 16, S]
    idx16 = np.tile(idx16, (1, 8, 1))                    # [NCH, 128, S]
    idx16 = np.ascontiguousarray(
        idx16.transpose(1, 0, 2).reshape(P, NCH * S)
    )
    return idx16, counts.reshape(1, NCH), u_rank, uvals, valid_u


def kernel(indices: np.ndarray, weight: np.ndarray) -> np.ndarray:
    global _NC_CACHE
    from concourse.bass_utils import run_bass_kernel_spmd

    indices = np.asarray(indices)
    weight = np.ascontiguousarray(np.asarray(weight, dtype=np.float32))
    assert indices.shape == (B, L), indices.shape
    assert weight.shape == (V, D), weight.shape

    if _NC_CACHE is None:
        _NC_CACHE = _build_module()
    nc = _NC_CACHE

    gflat = indices.reshape(-1).astype(np.int64)
    g_order = np.argsort(gflat, kind="stable")           # routes + sorts
    sv = gflat[g_order]                                  # ascending values
    bounds = np.searchsorted(sv, np.arange(N_CORES + 1) * SHARD)

    in_maps = []
    metas = []
    for i in range(N_CORES):
        lo, hi = int(bounds[i]), int(bounds[i + 1])
        local = sv[lo:hi] - i * SHARD
        idx16, counts, u_rank, uvals, valid_u = _prep_core(local)
        in_maps.append({
            "idxs": idx16,
            "counts": counts,
            "weight": weight[i * SHARD:(i + 1) * SHARD],
        })
        metas.append((lo, hi, u_rank, uvals, valid_u, counts))

    res = run_bass_kernel_spmd(nc, in_maps, core_ids=list(range(N_CORES)))

    slot = np.arange(CHUNK)
    result = np.empty((N_FLAT, D), dtype=np.float32)
    for i in range(N_CORES):
        lo, hi, u_rank, uvals, valid_u, counts = metas[i]
        if hi == lo:
            continue
        arr = res.results[i]["out"].reshape(NCH, P, J, D)
        rows = arr[:, slot % 128, slot // 128, :]        # [NCH, CHUNK, D]
        n_u = len(uvals)
        # device rows are in unique-rank order: gathered uniques form, per
        # chunk, a compacted prefix
        full_u = np.empty((n_u, D), dtype=np.float32)
        for c in range(NCH):
            lo_u = c * CHUNK
            hi_u = min((c + 1) * CHUNK, n_u)
            if lo_u >= n_u:
                break
            vm = valid_u[lo_u:hi_u]
            kreal = int(vm.sum())
            if kreal:
                full_u[lo_u + vm.nonzero()[0]] = rows[c, :kreal]
        if not valid_u.all():                            # spills: host gather
            miss = (~valid_u).nonzero()[0]
            full_u[miss] = weight[i * SHARD + uvals[miss]]
        result[g_order[lo:hi]] = full_u[u_rank]

    return result.reshape(B, L, D)


# revision 17
# speedup vs baseline: 1.0881x; 1.0881x over previous
"""Embedding gather (DirectCXLEmbedding) on 8 TRN2 NeuronCores.

Design (vocab-sharded + dedup + pair-coalesced int16 SWDGE gather):

1. Vocab (table) sharding: core i owns table rows [i*125000, (i+1)*125000)
   and handles the indices landing in its shard (~102,400 of the global
   819,200 for uniform inputs).  The host routes indices to owner cores by
   sorting them once; the "all-to-all" of classic vocab-sharded embeddings
   is free because kernel() owns full inputs and outputs anyway.  Each core
   only receives its 32 MB table slice.

2. Dedup: at 0.82 draws/row, ~32% of a core's sorted indices are
   duplicates.  The device gathers each unique row once (~70,000 rows/core);
   the host expands duplicates during the same fancy-index that inverts the
   sort.

3. Pair coalescing: unique rows are dense in the shard (~0.56/row), so
   ~56% of them sit in even-aligned adjacent pairs (rows 2k, 2k+1 both
   needed).  Those move as ONE 512-B gather element (elem_size=128 f32),
   halving their descriptor count and clearing the sub-512B DMA penalty on
   both the HBM read and SBUF write side.  Remaining rows go through a
   256-B single-row stream.  ~50K descriptors/core instead of 102K naive.

4. Gather engine: GPSIMD `dma_gather` (InstDMAGatherAnt, SWDGE) gathers up
   to 1024 elements per instruction (HW limit found empirically; >1024
   crashes the device) by int16 index.  Each stream is cut into chunks of
   <=1024 sorted elements; chunk c reads from a STATIC 32,768-row window
   based at the expected rank-quantile minus margin, so chunk-local indices
   fit int16 with large slack.  Out-of-window elements (non-uniform inputs)
   spill to a host-side numpy gather — zero spills for the target workload.

5. Device pipeline: per chunk, one dma_gather (valid count via a runtime
   register loaded from an input tensor) into an SBUF staging slot, then a
   contiguous HWDGE store from SP.  Gathers (GPSIMD/SWDGE) and stores
   (SP/HWDGE) overlap; staging slots rotate over NBUF per-slot semaphore
   pairs (a DMA's "+16" is 16 independent +1s from the SDMA engines, so a
   semaphore is only safely waitable with a single DMA in flight on it).
"""

import numpy as np

# Problem constants (hardcoded per harness contract).
B, L = 16384, 50
V, D = 1_000_000, 64
N_CORES = 8
P = 128
N_FLAT = B * L                            # 819,200 total gathers

SHARD = V // N_CORES                      # 125,000 table rows per core
CHUNK = 1024                              # max num_idxs per dma_gather
S = CHUNK // 16                           # int16 idx columns per chunk (64)
J = CHUNK // 128                          # elements per partition per chunk (8)
WIN = 1 << 15                             # int16 window (32768 rows)

NCHP = 21                                 # pair chunks  (21,504 pair capacity)
NCHS = 32                                 # single chunks (32,768 capacity)
NCHT = NCHP + NCHS                        # 53 gather instructions per core
CAP_P = NCHP * CHUNK
CAP_S = NCHS * CHUNK

PAIR_RANGE = SHARD // 2                   # pair-unit address space (62,500)
WIN_P = WIN // 2                          # window in pair units (16,384)
_E_PAIRS = 19_500                         # expected pairs per core
_E_SNGL = 30_800                          # expected singles per core

BASES_P = np.clip(
    (np.arange(NCHP) * CHUNK * PAIR_RANGE) // _E_PAIRS - 3_000,
    0,
    PAIR_RANGE - WIN_P,
).astype(np.int64)
BASES_S = np.clip(
    (np.arange(NCHS) * CHUNK * SHARD) // _E_SNGL - 6_000,
    0,
    SHARD - WIN,
).astype(np.int64)

NBUF = 16                                 # staging slots (4 KB/partition each)
SLOT = J * 2 * D                          # slot stride in f32 (pair-chunk size)


def _build_module():
    from contextlib import ExitStack

    import concourse.bacc as bacc
    import concourse.mybir as mybir

    nc = bacc.Bacc()

    idxs = nc.dram_tensor("idxs", [P, NCHT * S], mybir.dt.int16, kind="ExternalInput")
    counts = nc.dram_tensor("counts", [1, NCHT], mybir.dt.int32, kind="ExternalInput")
    weight = nc.dram_tensor("weight", [SHARD, D], mybir.dt.float32, kind="ExternalInput")
    out_p = nc.dram_tensor(
        "out_p", [NCHP, P, J * 2 * D], mybir.dt.float32, kind="ExternalOutput"
    )
    out_s = nc.dram_tensor(
        "out_s", [NCHS, P, J * D], mybir.dt.float32, kind="ExternalOutput"
    )

    with ExitStack() as ctx:
        idx_sb = ctx.enter_context(nc.sbuf_tensor([P, NCHT * S], mybir.dt.int16))
        cnt_sb = ctx.enter_context(nc.sbuf_tensor([1, NCHT], mybir.dt.int32))
        stage = ctx.enter_context(
            nc.sbuf_tensor([P, NBUF * SLOT], mybir.dt.float32)
        )
        ld_sem = ctx.enter_context(nc.semaphore("ld_sem"))
        ig_sems = [
            ctx.enter_context(nc.semaphore(f"ig{t}")) for t in range(NBUF)
        ]
        st_sems = [
            ctx.enter_context(nc.semaphore(f"st{t}")) for t in range(NBUF)
        ]
        vz_sem = ctx.enter_context(nc.semaphore("vz_sem"))
        cnt_reg = ctx.enter_context(nc.gpsimd.register("cnt_reg"))
        block = ctx.enter_context(nc.Block())

        @block.vector
        def _(v):
            # init staging once: -1-padded gather lanes are skipped by the HW,
            # so stores would otherwise move uninitialized SBUF
            v.memset(stage[:], 0.0).then_inc(vz_sem, 1)

        @block.gpsimd
        def _(g):
            g.wait_ge(vz_sem, 1)
            g.dma_start(out=idx_sb[:], in_=idxs[:]).then_inc(ld_sem, 16)
            g.wait_ge(ld_sem, 16)
            g.dma_start(out=cnt_sb[:], in_=counts[:]).then_inc(ld_sem, 16)
            g.wait_ge(ld_sem, 32)
            for c in range(NCHT):
                slot = c % NBUF
                if c >= NBUF:
                    # staging slot must have been stored out (same-lane store)
                    g.wait_ge(st_sems[slot], 16 * (c // NBUF))
                g.reg_load(cnt_reg, cnt_sb[0:1, c:c + 1])
                if c < NCHP:  # pair chunk: 512-B elements (two rows each)
                    base = int(BASES_P[c]) * 2
                    in_ap = weight[base:base + WIN, :].rearrange(
                        "(a two) d -> a (two d)", two=2
                    )
                    out_ap = stage[:, slot * SLOT:(slot + 1) * SLOT].rearrange(
                        "p (j d) -> p j d", d=2 * D
                    )
                    elem = 2 * D
                else:         # single chunk: 256-B elements
                    base = int(BASES_S[c - NCHP])
                    in_ap = weight[base:base + WIN, :]
                    out_ap = stage[
                        :, slot * SLOT:slot * SLOT + J * D
                    ].rearrange("p (j d) -> p j d", d=D)
                    elem = D
                g.dma_gather(
                    out_ap=out_ap,
                    in_ap=in_ap,
                    idxs_ap=idx_sb[:, c * S:(c + 1) * S],
                    num_idxs=CHUNK,
                    num_idxs_reg=cnt_reg,
                    elem_size=elem,
                ).then_inc(ig_sems[slot], 16)

        @block.sync
        def _(s):
            for c in range(NCHT):
                slot = c % NBUF
                s.wait_ge(ig_sems[slot], 16 * (c // NBUF + 1))
                if c < NCHP:
                    s.dma_start(
                        out=out_p[c, :, :],
                        in_=stage[:, slot * SLOT:(slot + 1) * SLOT],
                    ).then_inc(st_sems[slot], 16)
                else:
                    s.dma_start(
                        out=out_s[c - NCHP, :, :],
                        in_=stage[:, slot * SLOT:slot * SLOT + J * D],
                    ).then_inc(st_sems[slot], 16)
            for c in range(NCHT - NBUF, NCHT):
                slot = c % NBUF
                s.wait_ge(st_sems[slot], 16 * (c // NBUF + 1))

    nc.compile()
    return nc


_NC_CACHE = None


def _chunk_stream(vals: np.ndarray, bases: np.ndarray, nch: int, win: int):
    """Pack sorted element values into nch chunks of CHUNK int16 slots.

    Returns (buf [nch, CHUNK] int16, counts [nch] int32, valid mask over
    vals' ranks — True iff gathered on device)."""
    cap = nch * CHUNK
    n = len(vals)
    take = min(n, cap)
    pad = np.full(cap, -1, dtype=np.int64)
    pad[:take] = vals[:take]
    chunks = pad.reshape(nch, CHUNK)
    rel = chunks - bases[:, None]
    in_win = (rel >= 0) & (rel < win) & (chunks >= 0)

    buf = np.full((nch, CHUNK), -1, dtype=np.int16)
    counts = np.empty(nch, dtype=np.int32)
    nval = in_win.sum(axis=1)
    for c in range(nch):
        buf[c, :nval[c]] = rel[c][in_win[c]].astype(np.int16)
        if nval[c] == 0:
            buf[c, 0] = 0                                # dummy; discarded
        counts[c] = max(int(nval[c]), 1)

    valid = np.zeros(n, dtype=bool)
    valid[:take] = in_win.reshape(-1)[:take]
    return buf, counts, valid, nval


def _wrap16(buf: np.ndarray) -> np.ndarray:
    """[nch, CHUNK] -> 16-partition-wrapped, 8x-replicated [P, nch*S]."""
    nch = buf.shape[0]
    idx16 = buf.reshape(nch, S, 16).transpose(0, 2, 1)   # [nch, 16, S]
    idx16 = np.tile(idx16, (1, 8, 1))                    # [nch, 128, S]
    return np.ascontiguousarray(idx16.transpose(1, 0, 2).reshape(P, nch * S))


def kernel(indices: np.ndarray, weight: np.ndarray) -> np.ndarray:
    global _NC_CACHE
    from concourse.bass_utils import run_bass_kernel_spmd

    indices = np.asarray(indices)
    weight = np.ascontiguousarray(np.asarray(weight, dtype=np.float32))
    assert indices.shape == (B, L), indices.shape
    assert weight.shape == (V, D), weight.shape

    if _NC_CACHE is None:
        _NC_CACHE = _build_module()
    nc = _NC_CACHE

    gflat = indices.reshape(-1).astype(np.int64)
    g_order = np.argsort(gflat, kind="stable")           # routes + sorts
    sv = gflat[g_order]                                  # ascending values
    bounds = np.searchsorted(sv, np.arange(N_CORES + 1) * SHARD)

    in_maps = []
    metas = []
    for i in range(N_CORES):
        lo, hi = int(bounds[i]), int(bounds[i + 1])
        local = sv[lo:hi] - i * SHARD
        n = len(local)
        if n == 0:
            u = np.empty(0, np.int64)
            u_rank = np.empty(0, np.int64)
        else:
            newv = np.empty(n, dtype=bool)
            newv[0] = True
            np.not_equal(local[1:], local[:-1], out=newv[1:])
            u_rank = np.cumsum(newv) - 1                 # sorted rank -> u rank
            u = local[newv]                              # sorted unique values
        n_u = len(u)

        # even-aligned adjacent pairs
        nxt = np.zeros(n_u, dtype=bool)
        if n_u > 1:
            nxt[:-1] = u[1:] == u[:-1] + 1
        pairstart = (u % 2 == 0) & nxt
        member = pairstart.copy()
        member[1:] |= pairstart[:-1]
        pvals = u[pairstart] >> 1                        # pair-unit values
        svals = u[~member]
        pr = pairstart.nonzero()[0]                      # u-rank of pair start
        sr = (~member).nonzero()[0]                      # u-rank of singles

        buf_p, cnt_p, val_p, nval_p = _chunk_stream(pvals, BASES_P, NCHP, WIN_P)
        buf_s, cnt_s, val_s, nval_s = _chunk_stream(svals, BASES_S, NCHS, WIN)

        idx16 = np.concatenate(
            [_wrap16(buf_p), _wrap16(buf_s)], axis=1
        )
        counts = np.concatenate([cnt_p, cnt_s]).reshape(1, NCHT)
        in_maps.append({
            "idxs": idx16,
            "counts": counts,
            "weight": weight[i * SHARD:(i + 1) * SHARD],
        })
        metas.append((lo, hi, u, u_rank, pr, sr, val_p, val_s, nval_p, nval_s))

    res = run_bass_kernel_spmd(nc, in_maps, core_ids=list(range(N_CORES)))

    result = np.empty((N_FLAT, D), dtype=np.float32)
    for i in range(N_CORES):
        lo, hi, u, u_rank, pr, sr, val_p, val_s, nval_p, nval_s = metas[i]
        if hi == lo:
            continue
        n_u = len(u)
        full_u = np.empty((n_u, D), dtype=np.float32)
        filled = np.zeros(n_u, dtype=bool)

        slot = np.arange(CHUNK)
        arr_p = res.results[i]["out_p"].reshape(NCHP, P, J, 2 * D)
        rows_p = arr_p[:, slot % 128, slot // 128, :]    # [NCHP, CHUNK, 2D]
        n_p = len(pr)
        for c in range(NCHP):
            lo_e, hi_e = c * CHUNK, min((c + 1) * CHUNK, n_p)
            if lo_e >= n_p:
                break
            vm = val_p[lo_e:hi_e]
            k = int(vm.sum())
            if k == 0:
                continue
            ru = pr[lo_e + vm.nonzero()[0]]              # pair-start u-ranks
            full_u[ru] = rows_p[c, :k, :D]
            full_u[ru + 1] = rows_p[c, :k, D:]
            filled[ru] = True
            filled[ru + 1] = True

        arr_s = res.results[i]["out_s"].reshape(NCHS, P, J, D)
        rows_s = arr_s[:, slot % 128, slot // 128, :]    # [NCHS, CHUNK, D]
        n_s = len(sr)
        for c in range(NCHS):
            lo_e, hi_e = c * CHUNK, min((c + 1) * CHUNK, n_s)
            if lo_e >= n_s:
                break
            vm = val_s[lo_e:hi_e]
            k = int(vm.sum())
            if k == 0:
                continue
            ru = sr[lo_e + vm.nonzero()[0]]
            full_u[ru] = rows_s[c, :k]
            filled[ru] = True

        if not filled.all():                             # spills: host gather
            miss = (~filled).nonzero()[0]
            full_u[miss] = weight[i * SHARD + u[miss]]
        result[g_order[lo:hi]] = full_u[u_rank]

    return result.reshape(B, L, D)


# revision 19
# speedup vs baseline: 1.1760x; 1.0808x over previous
"""Embedding gather (DirectCXLEmbedding) on 8 TRN2 NeuronCores.

Design (vocab-sharded + dedup + greedy pair-coalesced int16 SWDGE gather):

1. Vocab (table) sharding: core i owns table rows [i*125000, (i+1)*125000)
   and handles the indices landing in its shard (~102,400 of the global
   819,200 for uniform inputs).  The host routes indices to owner cores by
   sorting them once; the "all-to-all" of classic vocab-sharded embeddings
   is free because kernel() owns full inputs and outputs anyway.  Each core
   only receives its 32 MB table slice.

2. Dedup: at 0.82 draws/row, ~32% of a core's sorted indices are
   duplicates.  The device gathers each unique row once (~70,000 rows/core);
   the host expands duplicates during the same fancy-index that inverts the
   sort.

3. Greedy pair coalescing: unique rows are dense in the shard (~0.56/row).
   Greedy pairing of adjacent unique rows covers ~72% of them; each pair
   moves as ONE 512-B gather element (elem_size=128 f32), halving its
   descriptor count and clearing the sub-512B DMA penalty on both the HBM
   read and SBUF write side.  Pairs starting at even rows use the table
   viewed as [62500, 128]; pairs starting at odd rows use the same view
   shifted one row; leftovers go through a 256-B single-row stream.
   ~45K gather elements/core instead of 102K naive.

4. Gather engine: GPSIMD `dma_gather` (InstDMAGatherAnt, SWDGE) gathers up
   to 1024 elements per instruction (HW limit found empirically; >1024
   crashes the device) by int16 index.  Each stream is cut into chunks of
   <=1024 sorted elements; chunk c reads from a STATIC 32,768-row window
   based at the expected rank-quantile minus margin, so chunk-local indices
   fit int16 with large slack.  Out-of-window elements (non-uniform inputs)
   spill to a host-side numpy gather — zero spills for the target workload.

5. Device pipeline: per chunk, one dma_gather (valid count via a runtime
   register loaded from an input tensor) into an SBUF staging slot, then a
   contiguous HWDGE store from SP.  Gathers (GPSIMD/SWDGE) and stores
   (SP/HWDGE) overlap; staging slots rotate over NBUF per-slot semaphore
   pairs (a DMA's "+16" is 16 independent +1s from the SDMA engines, so a
   semaphore is only safely waitable with a single DMA in flight on it).
"""

import numpy as np

# Problem constants (hardcoded per harness contract).
B, L = 16384, 50
V, D = 1_000_000, 64
N_CORES = 8
P = 128
N_FLAT = B * L                            # 819,200 total gathers

SHARD = V // N_CORES                      # 125,000 table rows per core
CHUNK = 1024                              # max num_idxs per dma_gather
S = CHUNK // 16                           # int16 idx columns per chunk (64)
J = CHUNK // 128                          # elements per partition per chunk (8)
WIN = 1 << 15                             # int16 window (32768 rows)

# chunk counts per stream (sized to the uniform workload's per-core maxima
# plus ~5 sigma; out-of-capacity inputs spill to the host path)
NCH_E = 13                                # even-aligned pair chunks
NCH_O = 13                                # odd-aligned pair chunks
NCH_S = 20                                # single chunks
NCHT = NCH_E + NCH_O + NCH_S              # 46 gather instructions per core

PAIR_RANGE = SHARD // 2                   # pair-unit address space (62,500)
WIN_P = WIN // 2                          # window in pair units (16,384)
_E_PAIR = 12_600                          # expected pairs per alignment
_E_SNGL = 19_800                          # expected singles per core

BASES_E = np.clip(
    (np.arange(NCH_E) * CHUNK * PAIR_RANGE) // _E_PAIR - 3_000,
    0,
    PAIR_RANGE - WIN_P,
).astype(np.int64)
BASES_O = np.clip(
    (np.arange(NCH_O) * CHUNK * PAIR_RANGE) // _E_PAIR - 3_000,
    0,
    PAIR_RANGE - WIN_P - 1,
).astype(np.int64)
BASES_S = np.clip(
    (np.arange(NCH_S) * CHUNK * SHARD) // _E_SNGL - 6_000,
    0,
    SHARD - WIN,
).astype(np.int64)

NBUF = 16                                 # staging slots (4 KB/partition each)
SLOT = J * 2 * D                          # slot stride in f32 (pair-chunk size)


def _build_module():
    from contextlib import ExitStack

    import concourse.bacc as bacc
    import concourse.mybir as mybir

    nc = bacc.Bacc()

    idxs = nc.dram_tensor("idxs", [P, NCHT * S], mybir.dt.int16, kind="ExternalInput")
    counts = nc.dram_tensor("counts", [1, NCHT], mybir.dt.int32, kind="ExternalInput")
    weight = nc.dram_tensor("weight", [SHARD, D], mybir.dt.float32, kind="ExternalInput")
    out_p = nc.dram_tensor(
        "out_p", [NCH_E + NCH_O, P, J * 2 * D], mybir.dt.float32,
        kind="ExternalOutput",
    )
    out_s = nc.dram_tensor(
        "out_s", [NCH_S, P, J * D], mybir.dt.float32, kind="ExternalOutput"
    )

    with ExitStack() as ctx:
        idx_sb = ctx.enter_context(nc.sbuf_tensor([P, NCHT * S], mybir.dt.int16))
        cnt_sb = ctx.enter_context(nc.sbuf_tensor([1, NCHT], mybir.dt.int32))
        stage = ctx.enter_context(
            nc.sbuf_tensor([P, NBUF * SLOT], mybir.dt.float32)
        )
        ld_sem = ctx.enter_context(nc.semaphore("ld_sem"))
        ig_sems = [
            ctx.enter_context(nc.semaphore(f"ig{t}")) for t in range(NBUF)
        ]
        st_sems = [
            ctx.enter_context(nc.semaphore(f"st{t}")) for t in range(NBUF)
        ]
        vz_sem = ctx.enter_context(nc.semaphore("vz_sem"))
        cnt_reg = ctx.enter_context(nc.gpsimd.register("cnt_reg"))
        block = ctx.enter_context(nc.Block())

        @block.vector
        def _(v):
            # init staging once: -1-padded gather lanes are skipped by the HW,
            # so stores would otherwise move uninitialized SBUF
            v.memset(stage[:], 0.0).then_inc(vz_sem, 1)

        @block.gpsimd
        def _(g):
            g.wait_ge(vz_sem, 1)
            g.dma_start(out=idx_sb[:], in_=idxs[:]).then_inc(ld_sem, 16)
            g.wait_ge(ld_sem, 16)
            g.dma_start(out=cnt_sb[:], in_=counts[:]).then_inc(ld_sem, 16)
            g.wait_ge(ld_sem, 32)
            for c in range(NCHT):
                slot = c % NBUF
                if c >= NBUF:
                    # staging slot must have been stored out (same-lane store)
                    g.wait_ge(st_sems[slot], 16 * (c // NBUF))
                g.reg_load(cnt_reg, cnt_sb[0:1, c:c + 1])
                if c < NCH_E + NCH_O:     # pair chunk: 512-B elements
                    if c < NCH_E:
                        row0 = int(BASES_E[c]) * 2
                    else:
                        row0 = int(BASES_O[c - NCH_E]) * 2 + 1
                    in_ap = weight[row0:row0 + WIN, :].rearrange(
                        "(a two) d -> a (two d)", two=2
                    )
                    out_ap = stage[:, slot * SLOT:(slot + 1) * SLOT].rearrange(
                        "p (j d) -> p j d", d=2 * D
                    )
                    elem = 2 * D
                else:                     # single chunk: 256-B elements
                    row0 = int(BASES_S[c - NCH_E - NCH_O])
                    in_ap = weight[row0:row0 + WIN, :]
                    out_ap = stage[
                        :, slot * SLOT:slot * SLOT + J * D
                    ].rearrange("p (j d) -> p j d", d=D)
                    elem = D
                g.dma_gather(
                    out_ap=out_ap,
                    in_ap=in_ap,
                    idxs_ap=idx_sb[:, c * S:(c + 1) * S],
                    num_idxs=CHUNK,
                    num_idxs_reg=cnt_reg,
                    elem_size=elem,
                ).then_inc(ig_sems[slot], 16)

        @block.sync
        def _(s):
            for c in range(NCHT):
                slot = c % NBUF
                s.wait_ge(ig_sems[slot], 16 * (c // NBUF + 1))
                if c < NCH_E + NCH_O:
                    s.dma_start(
                        out=out_p[c, :, :],
                        in_=stage[:, slot * SLOT:(slot + 1) * SLOT],
                    ).then_inc(st_sems[slot], 16)
                else:
                    s.dma_start(
                        out=out_s[c - NCH_E - NCH_O, :, :],
                        in_=stage[:, slot * SLOT:slot * SLOT + J * D],
                    ).then_inc(st_sems[slot], 16)
            for c in range(NCHT - NBUF, NCHT):
                slot = c % NBUF
                s.wait_ge(st_sems[slot], 16 * (c // NBUF + 1))

    nc.compile()
    return nc


_NC_CACHE = None


def _chunk_stream(vals: np.ndarray, bases: np.ndarray, nch: int, win: int):
    """Pack sorted element values into nch chunks of CHUNK int16 slots.

    Returns (buf [nch, CHUNK] int16, counts [nch] int32, valid mask over
    vals' ranks — True iff gathered on device)."""
    cap = nch * CHUNK
    n = len(vals)
    take = min(n, cap)
    pad = np.full(cap, -1, dtype=np.int64)
    pad[:take] = vals[:take]
    chunks = pad.reshape(nch, CHUNK)
    rel = chunks - bases[:, None]
    in_win = (rel >= 0) & (rel < win) & (chunks >= 0)

    buf = np.full((nch, CHUNK), -1, dtype=np.int16)
    counts = np.empty(nch, dtype=np.int32)
    nval = in_win.sum(axis=1)
    for c in range(nch):
        buf[c, :nval[c]] = rel[c][in_win[c]].astype(np.int16)
        if nval[c] == 0:
            buf[c, 0] = 0                                # dummy; discarded
        counts[c] = max(int(nval[c]), 1)

    valid = np.zeros(n, dtype=bool)
    valid[:take] = in_win.reshape(-1)[:take]
    return buf, counts, valid


def _wrap16(buf: np.ndarray) -> np.ndarray:
    """[nch, CHUNK] -> 16-partition-wrapped, 8x-replicated [P, nch*S]."""
    nch = buf.shape[0]
    idx16 = buf.reshape(nch, S, 16).transpose(0, 2, 1)   # [nch, 16, S]
    idx16 = np.tile(idx16, (1, 8, 1))                    # [nch, 128, S]
    return np.ascontiguousarray(idx16.transpose(1, 0, 2).reshape(P, nch * S))


def _scatter_stream(full_u, filled, rows, ranks, valid, nch, two):
    """Write device rows of one stream into full_u at the streams' u-ranks.

    rows: [nch, CHUNK, elem_D]; ranks: u-rank per stream element; valid:
    gathered mask per stream element (device rows form compacted prefixes
    per chunk)."""
    n = len(ranks)
    for c in range(nch):
        lo_e, hi_e = c * CHUNK, min((c + 1) * CHUNK, n)
        if lo_e >= n:
            break
        vm = valid[lo_e:hi_e]
        k = int(vm.sum())
        if k == 0:
            continue
        ru = ranks[lo_e + vm.nonzero()[0]]
        if two:
            full_u[ru] = rows[c, :k, :D]
            full_u[ru + 1] = rows[c, :k, D:]
            filled[ru] = True
            filled[ru + 1] = True
        else:
            full_u[ru] = rows[c, :k]
            filled[ru] = True


def kernel(indices: np.ndarray, weight: np.ndarray) -> np.ndarray:
    global _NC_CACHE
    from concourse.bass_utils import run_bass_kernel_spmd

    indices = np.asarray(indices)
    weight = np.ascontiguousarray(np.asarray(weight, dtype=np.float32))
    assert indices.shape == (B, L), indices.shape
    assert weight.shape == (V, D), weight.shape

    if _NC_CACHE is None:
        _NC_CACHE = _build_module()
    nc = _NC_CACHE

    gflat = indices.reshape(-1).astype(np.int64)
    g_order = np.argsort(gflat, kind="stable")           # routes + sorts
    sv = gflat[g_order]                                  # ascending values
    bounds = np.searchsorted(sv, np.arange(N_CORES + 1) * SHARD)

    in_maps = []
    metas = []
    for i in range(N_CORES):
        lo, hi = int(bounds[i]), int(bounds[i + 1])
        local = sv[lo:hi] - i * SHARD
        n = len(local)
        if n == 0:
            u = np.empty(0, np.int64)
            u_rank = np.empty(0, np.int64)
        else:
            newv = np.empty(n, dtype=bool)
            newv[0] = True
            np.not_equal(local[1:], local[:-1], out=newv[1:])
            u_rank = np.cumsum(newv) - 1                 # sorted rank -> u rank
            u = local[newv]                              # sorted unique values
        n_u = len(u)

        # greedy pairing of adjacent unique rows (within runs)
        adj_next = np.zeros(n_u, dtype=bool)
        if n_u > 1:
            adj_next[:-1] = u[1:] == u[:-1] + 1
        adj_prev = np.zeros(n_u, dtype=bool)
        adj_prev[1:] = adj_next[:-1]
        run_start = ~adj_prev
        ar = np.arange(n_u)
        first = np.maximum.accumulate(np.where(run_start, ar, -1))
        pairstart = ((ar - first) % 2 == 0) & adj_next
        member = pairstart.copy()
        member[1:] |= pairstart[:-1]

        even_ps = pairstart & (u % 2 == 0)
        odd_ps = pairstart & (u % 2 == 1)
        e_vals = u[even_ps] >> 1                         # pair units
        o_vals = (u[odd_ps] - 1) >> 1
        s_vals = u[~member]
        e_ranks = even_ps.nonzero()[0]
        o_ranks = odd_ps.nonzero()[0]
        s_ranks = (~member).nonzero()[0]

        buf_e, cnt_e, val_e = _chunk_stream(e_vals, BASES_E, NCH_E, WIN_P)
        buf_o, cnt_o, val_o = _chunk_stream(o_vals, BASES_O, NCH_O, WIN_P)
        buf_s, cnt_s, val_s = _chunk_stream(s_vals, BASES_S, NCH_S, WIN)

        idx16 = np.concatenate(
            [_wrap16(buf_e), _wrap16(buf_o), _wrap16(buf_s)], axis=1
        )
        counts = np.concatenate([cnt_e, cnt_o, cnt_s]).reshape(1, NCHT)
        in_maps.append({
            "idxs": idx16,
            "counts": counts,
            "weight": weight[i * SHARD:(i + 1) * SHARD],
        })
        metas.append((lo, hi, u, u_rank,
                      e_ranks, o_ranks, s_ranks, val_e, val_o, val_s))

    res = run_bass_kernel_spmd(nc, in_maps, core_ids=list(range(N_CORES)))

    slot = np.arange(CHUNK)
    result = np.empty((N_FLAT, D), dtype=np.float32)
    for i in range(N_CORES):
        (lo, hi, u, u_rank,
         e_ranks, o_ranks, s_ranks, val_e, val_o, val_s) = metas[i]
        if hi == lo:
            continue
        n_u = len(u)
        full_u = np.empty((n_u, D), dtype=np.float32)
        filled = np.zeros(n_u, dtype=bool)

        arr_p = res.results[i]["out_p"].reshape(NCH_E + NCH_O, P, J, 2 * D)
        rows_p = arr_p[:, slot % 128, slot // 128, :]    # [.., CHUNK, 2D]
        _scatter_stream(full_u, filled, rows_p[:NCH_E], e_ranks, val_e,
                        NCH_E, two=True)
        _scatter_stream(full_u, filled, rows_p[NCH_E:], o_ranks, val_o,
                        NCH_O, two=True)

        arr_s = res.results[i]["out_s"].reshape(NCH_S, P, J, D)
        rows_s = arr_s[:, slot % 128, slot // 128, :]    # [NCH_S, CHUNK, D]
        _scatter_stream(full_u, filled, rows_s, s_ranks, val_s,
                        NCH_S, two=False)

        if not filled.all():                             # spills: host gather
            miss = (~filled).nonzero()[0]
            full_u[miss] = weight[i * SHARD + u[miss]]
        result[g_order[lo:hi]] = full_u[u_rank]

    return result.reshape(B, L, D)


# revision 25
# speedup vs baseline: 1.3587x; 1.1554x over previous
"""Embedding gather (DirectCXLEmbedding) on 8 TRN2 NeuronCores.

Design (vocab-sharded + dedup + greedy pair-coalesced int16 SWDGE gather):

1. Vocab (table) sharding: core i owns table rows [i*125000, (i+1)*125000)
   and handles the indices landing in its shard (~102,400 of the global
   819,200 for uniform inputs).  The host routes indices to owner cores by
   sorting them once; the "all-to-all" of classic vocab-sharded embeddings
   is free because kernel() owns full inputs and outputs anyway.  Each core
   only receives its 32 MB table slice.

2. Dedup: at 0.82 draws/row, ~32% of a core's sorted indices are
   duplicates.  The device gathers each unique row once (~70,000 rows/core);
   the host expands duplicates during the same fancy-index that inverts the
   sort.

3. Greedy pair coalescing: unique rows are dense in the shard (~0.56/row).
   Greedy pairing of adjacent unique rows covers ~72% of them; each pair
   moves as ONE 512-B gather element (elem_size=128 f32), halving its
   descriptor count and clearing the sub-512B DMA penalty on both the HBM
   read and SBUF write side.  Pairs starting at even rows use the table
   viewed as [62500, 128]; pairs starting at odd rows use the same view
   shifted one row; leftovers go through a 256-B single-row stream.
   ~45K gather elements/core instead of 102K naive.

4. Gather engine: GPSIMD `dma_gather` (InstDMAGatherAnt, SWDGE) gathers up
   to 1024 elements per instruction (HW limit found empirically; >1024
   crashes the device) by int16 index.  Each stream is cut into chunks of
   <=1024 sorted elements; chunk c reads from a STATIC 32,768-row window
   based at the expected rank-quantile minus margin, so chunk-local indices
   fit int16 with large slack.  Out-of-window elements (non-uniform inputs)
   spill to a host-side numpy gather — zero spills for the target workload.

5. Device pipeline: per chunk, one full-capacity dma_gather (unused slots
   carry a dummy in-window index 0, so every staging lane is written — no
   staging memset, no valid-count plumbing) into an SBUF staging slot, then
   a contiguous HWDGE store from SP.  Gathers (GPSIMD/SWDGE) and stores
   (SP/HWDGE) overlap; staging slots rotate over NBUF per-slot semaphore
   pairs (a DMA's "+16" is 16 independent +1s from the SDMA engines, so a
   semaphore is only safely waitable with a single DMA in flight on it).
"""

import numpy as np

# Problem constants (hardcoded per harness contract).
B, L = 16384, 50
V, D = 1_000_000, 64
N_CORES = 8
P = 128
N_FLAT = B * L                            # 819,200 total gathers

SHARD = V // N_CORES                      # 125,000 table rows per core
CHUNK = 1024                              # max num_idxs per dma_gather
S = CHUNK // 16                           # int16 idx columns per chunk (64)
J = CHUNK // 128                          # elements per partition per chunk (8)
WIN = 1 << 15                             # int16 window (32768 rows)

# chunk counts per stream (sized to the uniform workload's per-core maxima
# plus ~5 sigma; out-of-capacity inputs spill to the host path)
NCH_E = 13                                # even-aligned pair chunks
NCH_O = 13                                # odd-aligned pair chunks
NCH_S = 20                                # single chunks
NCHT = NCH_E + NCH_O + NCH_S              # 46 gather instructions per core

PAIR_RANGE = SHARD // 2                   # pair-unit address space (62,500)
WIN_P = WIN // 2                          # window in pair units (16,384)
_E_PAIR = 12_600                          # expected pairs per alignment
_E_SNGL = 19_800                          # expected singles per core

BASES_E = np.clip(
    (np.arange(NCH_E) * CHUNK * PAIR_RANGE) // _E_PAIR - 3_000,
    0,
    PAIR_RANGE - WIN_P,
).astype(np.int64)
BASES_O = np.clip(
    (np.arange(NCH_O) * CHUNK * PAIR_RANGE) // _E_PAIR - 3_000,
    0,
    PAIR_RANGE - WIN_P - 1,
).astype(np.int64)
BASES_S = np.clip(
    (np.arange(NCH_S) * CHUNK * SHARD) // _E_SNGL - 6_000,
    0,
    SHARD - WIN,
).astype(np.int64)

NBUF = 16                                 # staging slots (4 KB/partition each)
SLOT = J * 2 * D                          # slot stride in f32 (pair-chunk size)


def _build_module():
    from contextlib import ExitStack

    import concourse.bacc as bacc
    import concourse.mybir as mybir

    nc = bacc.Bacc()

    idxs = nc.dram_tensor("idxs", [P, NCHT * S], mybir.dt.int16, kind="ExternalInput")
    weight = nc.dram_tensor("weight", [SHARD, D], mybir.dt.float32, kind="ExternalInput")
    out_p = nc.dram_tensor(
        "out_p", [NCH_E + NCH_O, P, J * 2 * D], mybir.dt.float32,
        kind="ExternalOutput",
    )
    out_s = nc.dram_tensor(
        "out_s", [NCH_S, P, J * D], mybir.dt.float32, kind="ExternalOutput"
    )

    with ExitStack() as ctx:
        idx_sb = ctx.enter_context(nc.sbuf_tensor([P, NCHT * S], mybir.dt.int16))
        stage = ctx.enter_context(
            nc.sbuf_tensor([P, NBUF * SLOT], mybir.dt.float32)
        )
        ld_sem = ctx.enter_context(nc.semaphore("ld_sem"))
        ig_sems = [
            ctx.enter_context(nc.semaphore(f"ig{t}")) for t in range(NBUF)
        ]
        st_sems = [
            ctx.enter_context(nc.semaphore(f"st{t}")) for t in range(NBUF)
        ]
        block = ctx.enter_context(nc.Block())

        @block.gpsimd
        def _(g):
            g.dma_start(out=idx_sb[:], in_=idxs[:]).then_inc(ld_sem, 16)
            g.wait_ge(ld_sem, 16)
            for c in range(NCHT):
                slot = c % NBUF
                if c >= NBUF:
                    # staging slot must have been stored out (same-lane store)
                    g.wait_ge(st_sems[slot], 16 * (c // NBUF))
                if c < NCH_E + NCH_O:     # pair chunk: 512-B elements
                    if c < NCH_E:
                        row0 = int(BASES_E[c]) * 2
                    else:
                        row0 = int(BASES_O[c - NCH_E]) * 2 + 1
                    in_ap = weight[row0:row0 + WIN, :].rearrange(
                        "(a two) d -> a (two d)", two=2
                    )
                    out_ap = stage[:, slot * SLOT:(slot + 1) * SLOT].rearrange(
                        "p (j d) -> p j d", d=2 * D
                    )
                    elem = 2 * D
                else:                     # single chunk: 256-B elements
                    row0 = int(BASES_S[c - NCH_E - NCH_O])
                    in_ap = weight[row0:row0 + WIN, :]
                    out_ap = stage[
                        :, slot * SLOT:slot * SLOT + J * D
                    ].rearrange("p (j d) -> p j d", d=D)
                    elem = D
                g.dma_gather(
                    out_ap=out_ap,
                    in_ap=in_ap,
                    idxs_ap=idx_sb[:, c * S:(c + 1) * S],
                    num_idxs=CHUNK,
                    num_idxs_reg=CHUNK,
                    elem_size=elem,
                ).then_inc(ig_sems[slot], 16)

        @block.sync
        def _(s):
            for c in range(NCHT):
                slot = c % NBUF
                s.wait_ge(ig_sems[slot], 16 * (c // NBUF + 1))
                if c < NCH_E + NCH_O:
                    s.dma_start(
                        out=out_p[c, :, :],
                        in_=stage[:, slot * SLOT:(slot + 1) * SLOT],
                    ).then_inc(st_sems[slot], 16)
                else:
                    s.dma_start(
                        out=out_s[c - NCH_E - NCH_O, :, :],
                        in_=stage[:, slot * SLOT:slot * SLOT + J * D],
                    ).then_inc(st_sems[slot], 16)
            for c in range(NCHT - NBUF, NCHT):
                slot = c % NBUF
                s.wait_ge(st_sems[slot], 16 * (c // NBUF + 1))

    nc.compile()
    return nc


_NC_CACHE = None


def _chunk_stream(vals: np.ndarray, bases: np.ndarray, nch: int, win: int):
    """Pack sorted element values into nch chunks of CHUNK int16 slots.

    Unused slots get dummy index 0 (in-window), so the device always gathers
    full chunks and every staging lane is written.  Returns (buf [nch,
    CHUNK] int16, valid mask over vals' ranks — True iff gathered)."""
    cap = nch * CHUNK
    n = len(vals)
    take = min(n, cap)
    pad = np.full(cap, -1, dtype=np.int64)
    pad[:take] = vals[:take]
    chunks = pad.reshape(nch, CHUNK)
    rel = chunks - bases[:, None]
    in_win = (rel >= 0) & (rel < win) & (chunks >= 0)

    buf = np.zeros((nch, CHUNK), dtype=np.int16)         # dummy idx 0
    nval = in_win.sum(axis=1)
    for c in range(nch):
        buf[c, :nval[c]] = rel[c][in_win[c]].astype(np.int16)

    valid = np.zeros(n, dtype=bool)
    valid[:take] = in_win.reshape(-1)[:take]
    return buf, valid


def _wrap16(buf: np.ndarray) -> np.ndarray:
    """[nch, CHUNK] -> 16-partition-wrapped, 8x-replicated [P, nch*S]."""
    nch = buf.shape[0]
    idx16 = buf.reshape(nch, S, 16).transpose(0, 2, 1)   # [nch, 16, S]
    idx16 = np.tile(idx16, (1, 8, 1))                    # [nch, 128, S]
    return np.ascontiguousarray(idx16.transpose(1, 0, 2).reshape(P, nch * S))


def _scatter_stream(full_u, filled, rows, ranks, valid, nch, two):
    """Write device rows of one stream into full_u at the streams' u-ranks.

    rows: [nch, CHUNK, elem_D]; ranks: u-rank per stream element; valid:
    gathered mask per stream element (device rows form compacted prefixes
    per chunk)."""
    n = len(ranks)
    for c in range(nch):
        lo_e, hi_e = c * CHUNK, min((c + 1) * CHUNK, n)
        if lo_e >= n:
            break
        vm = valid[lo_e:hi_e]
        k = int(vm.sum())
        if k == 0:
            continue
        ru = ranks[lo_e + vm.nonzero()[0]]
        if two:
            full_u[ru] = rows[c, :k, :D]
            full_u[ru + 1] = rows[c, :k, D:]
            filled[ru] = True
            filled[ru + 1] = True
        else:
            full_u[ru] = rows[c, :k]
            filled[ru] = True


def kernel(indices: np.ndarray, weight: np.ndarray) -> np.ndarray:
    global _NC_CACHE
    from concourse.bass_utils import run_bass_kernel_spmd

    indices = np.asarray(indices)
    weight = np.ascontiguousarray(np.asarray(weight, dtype=np.float32))
    assert indices.shape == (B, L), indices.shape
    assert weight.shape == (V, D), weight.shape

    if _NC_CACHE is None:
        _NC_CACHE = _build_module()
    nc = _NC_CACHE

    gflat = indices.reshape(-1).astype(np.int64)
    g_order = np.argsort(gflat, kind="stable")           # routes + sorts
    sv = gflat[g_order]                                  # ascending values
    bounds = np.searchsorted(sv, np.arange(N_CORES + 1) * SHARD)

    in_maps = []
    metas = []
    for i in range(N_CORES):
        lo, hi = int(bounds[i]), int(bounds[i + 1])
        local = sv[lo:hi] - i * SHARD
        n = len(local)
        if n == 0:
            u = np.empty(0, np.int64)
            u_rank = np.empty(0, np.int64)
        else:
            newv = np.empty(n, dtype=bool)
            newv[0] = True
            np.not_equal(local[1:], local[:-1], out=newv[1:])
            u_rank = np.cumsum(newv) - 1                 # sorted rank -> u rank
            u = local[newv]                              # sorted unique values
        n_u = len(u)

        # greedy pairing of adjacent unique rows (within runs)
        adj_next = np.zeros(n_u, dtype=bool)
        if n_u > 1:
            adj_next[:-1] = u[1:] == u[:-1] + 1
        adj_prev = np.zeros(n_u, dtype=bool)
        adj_prev[1:] = adj_next[:-1]
        run_start = ~adj_prev
        ar = np.arange(n_u)
        first = np.maximum.accumulate(np.where(run_start, ar, -1))
        pairstart = ((ar - first) % 2 == 0) & adj_next
        member = pairstart.copy()
        member[1:] |= pairstart[:-1]

        even_ps = pairstart & (u % 2 == 0)
        odd_ps = pairstart & (u % 2 == 1)
        e_vals = u[even_ps] >> 1                         # pair units
        o_vals = (u[odd_ps] - 1) >> 1
        s_vals = u[~member]
        e_ranks = even_ps.nonzero()[0]
        o_ranks = odd_ps.nonzero()[0]
        s_ranks = (~member).nonzero()[0]

        buf_e, val_e = _chunk_stream(e_vals, BASES_E, NCH_E, WIN_P)
        buf_o, val_o = _chunk_stream(o_vals, BASES_O, NCH_O, WIN_P)
        buf_s, val_s = _chunk_stream(s_vals, BASES_S, NCH_S, WIN)

        idx16 = np.concatenate(
            [_wrap16(buf_e), _wrap16(buf_o), _wrap16(buf_s)], axis=1
        )
        in_maps.append({
            "idxs": idx16,
            "weight": weight[i * SHARD:(i + 1) * SHARD],
        })
        metas.append((lo, hi, u, u_rank,
                      e_ranks, o_ranks, s_ranks, val_e, val_o, val_s))

    res = run_bass_kernel_spmd(nc, in_maps, core_ids=list(range(N_CORES)))

    slot = np.arange(CHUNK)
    result = np.empty((N_FLAT, D), dtype=np.float32)
    for i in range(N_CORES):
        (lo, hi, u, u_rank,
         e_ranks, o_ranks, s_ranks, val_e, val_o, val_s) = metas[i]
        if hi == lo:
            continue
        n_u = len(u)
        full_u = np.empty((n_u, D), dtype=np.float32)
        filled = np.zeros(n_u, dtype=bool)

        arr_p = res.results[i]["out_p"].reshape(NCH_E + NCH_O, P, J, 2 * D)
        rows_p = arr_p[:, slot % 128, slot // 128, :]    # [.., CHUNK, 2D]
        _scatter_stream(full_u, filled, rows_p[:NCH_E], e_ranks, val_e,
                        NCH_E, two=True)
        _scatter_stream(full_u, filled, rows_p[NCH_E:], o_ranks, val_o,
                        NCH_O, two=True)

        arr_s = res.results[i]["out_s"].reshape(NCH_S, P, J, D)
        rows_s = arr_s[:, slot % 128, slot // 128, :]    # [NCH_S, CHUNK, D]
        _scatter_stream(full_u, filled, rows_s, s_ranks, val_s,
                        NCH_S, two=False)

        if not filled.all():                             # spills: host gather
            miss = (~filled).nonzero()[0]
            full_u[miss] = weight[i * SHARD + u[miss]]
        result[g_order[lo:hi]] = full_u[u_rank]

    return result.reshape(B, L, D)
